# revision 20
# baseline (speedup 1.0000x reference)
"""Trainium2 Bass kernel for nn_MultiHeadAttention_62835371540559.

Reference computation (B=2, S=2048, DM=1024, H=16, HD=64):
    kp = k @ Wk + bk; qp = q @ Wq + bq; vp = v @ Wv + bv   (per batch)
    scores[b,c,h,q] = sum_d kp[b,c,h,d] * qp[b,q,h,d]
    attn = softmax(scores, axis=q)          (no 1/sqrt(hd) scaling)
    out[b,c,h,d] = sum_q attn[b,c,h,q] * vp[b,q,h,d]
    result = out.reshape(B,S,H*HD) @ Wo + bo

Sharding: 8 cores = 2 batches x 4 head-groups (4 heads each); zero
duplicated FLOPs. Each core computes a partial output (its heads'
contribution to out @ Wo); the host sums the 4 partials per batch and
adds the bias terms (bo and bv @ Wo, exact because softmax rows sum
to 1; bk/bq applied on-device as per-partition biases).

Per-core dataflow (v2 — reversed PV, streaming windows):
  - k/q/v shipped fp16 slab-major [8, S, 128], x-bar transposed on load
    (chunked by column range so compute starts ~25us in instead of
    waiting for full slabs). K/Q projected transposed (KPT[j,i]) fp16;
    V projected natural into vp [q, head*(64+1)] bf16 with a ones
    column per head.
  - Scores ST[q,c] per head pair: two K=64 matmuls row-packed at
    partition bases 0/64; exp on ScalarE -> e tiles [128, 1024] bf16 in
    a deep SBUF pool (ACT is the pacing engine: 128 x ~1.04us).
  - PV reversed vs v1: lhsT = 128x128 block of E, rhs = [vp_h | 1]
    (N=65), accumulating OT[c-block, 65] over the 16 q-blocks.  Streams
    the SMALL operand instead of re-streaming E: 27.7us of PE instead
    of 54.6us.  Column 64 accumulates the softmax normalizer Z.
    Accumulators packed two c-blocks per PSUM bank ([128, 260]).
  - Posts per c-block (start of next window): reciprocal of Z (a
    per-partition scalar now), scale, PE-transpose [c,d]->[d,c] via
    identity (f32r, 1.5 cyc/row), copy into persistent opair, and (in
    pass 1) the 256-contraction output projection + store.
  - Emission is window-structured: window w = (pass, c-chunk); each of
    the 16 qb slots emits scores+exp plus interleaved PV quanta of the
    in-flight chunk, plus background units (V/t1 projections) so PE
    slack hides under the ACT stream.

Cost-model time target ~165us/core (v1 was 228us). SPMD on 8 cores.
"""

import sys
from collections import defaultdict

import numpy as np

if "/opt/trn_rl_repo" not in sys.path:
    sys.path.insert(0, "/opt/trn_rl_repo")

B, S_FULL, DM = 2, 2048, 1024
H, HD = 16, 64
HD1 = HD + 1
NCORES = 8
HPC = 4  # heads per core
JW = HPC * HD  # per-core projection width (256)


def build(nc, S=S_FULL, repeat=1):
    import concourse.mybir as mybir
    import concourse.tile as tile
    from concourse import masks

    dt = mybir.dt
    f16, f32, bf16 = dt.float16, dt.float32, dt.bfloat16
    f32r = dt.float32r
    P = 128
    KO = DM // P          # 8 k-slabs of the contraction dim
    NQB = S // P          # q blocks (16)
    CC = min(512, S // 4) # c-chunk width
    NCC = S // CC         # c chunks (4)
    NCB = CC // P         # c-blocks per chunk (4)
    IC = 256              # projection i-chunk
    NIC = S // IC
    MC = DM // 2
    W = 2 * NCC           # windows (8)
    EXP = mybir.ActivationFunctionType.Exp

    kx = nc.dram_tensor("kx", [KO, S, P], f16, kind="ExternalInput")
    qx = nc.dram_tensor("qx", [KO, S, P], f16, kind="ExternalInput")
    vx = nc.dram_tensor("vx", [KO, S, P], f16, kind="ExternalInput")
    wk = nc.dram_tensor("wk", [DM, JW], f16, kind="ExternalInput")
    wq = nc.dram_tensor("wq", [DM, JW], f16, kind="ExternalInput")
    wv = nc.dram_tensor("wv", [DM, JW], f16, kind="ExternalInput")
    wo = nc.dram_tensor("wo", [JW, DM], f32r, kind="ExternalInput")
    bk = nc.dram_tensor("bk", [JW], f32, kind="ExternalInput")
    bq = nc.dram_tensor("bq", [JW], f32, kind="ExternalInput")
    out = nc.dram_tensor("out", [S, DM], f32, kind="ExternalOutput")

    with tile.TileContext(nc) as tc:
      for _rep in range(repeat):
        with (
            tc.tile_pool(name="persist", bufs=1) as pp,
            tc.tile_pool(name="inputs", bufs=1) as inp,
            tc.tile_pool(name="ep", bufs=22) as ep,
            tc.tile_pool(name="work", bufs=1) as wrk,
            tc.tile_pool(name="stp", bufs=2, space="PSUM") as stp,
            tc.tile_pool(name="otp", bufs=1, space="PSUM") as otp,
            tc.tile_pool(name="psm", bufs=2, space="PSUM") as psm,
        ):
            kpt = [pp.tile([P, S], f16, tag=f"kpt{t}", name=f"kpt{t}") for t in range(2)]
            qpt = [pp.tile([P, S], f16, tag=f"qpt{t}", name=f"qpt{t}") for t in range(2)]
            vp = pp.tile([P, NQB, HPC * HD1], bf16, tag="vp")
            opair0 = pp.tile([P, S], f32r, tag="opair0")
            wo_sb = pp.tile([P, 2, DM], f32r, tag="wo")
            wk_sb = pp.tile([P, KO, JW], f16, tag="wk")
            wq_sb = pp.tile([P, KO, JW], f16, tag="wq")
            wv_sb = pp.tile([P, KO, JW], f16, tag="wv")
            bk_sb = pp.tile([P, 2], f32, tag="bk")
            bq_sb = pp.tile([P, 2], f32, tag="bq")
            ident = pp.tile([P, P], f32, tag="ident")
            masks.make_identity(nc, ident[:])

            # ones columns (col HD of each head's 65-wide group)
            vp4 = vp[:].rearrange("p q (h x) -> p q h x", h=HPC)
            ones1 = pp.tile([P, 1], bf16, tag="ones1")
            nc.vector.memset(ones1[:], 1.0)
            nc.vector.tensor_copy(
                vp4[:, :, :, HD : HD + 1],
                ones1[:, None, None, :].to_broadcast((P, NQB, HPC, 1)),
            )

            kxT = [inp.tile([P, S], f16, tag=f"kxT{ko}", name=f"kxT{ko}") for ko in range(KO)]
            qxT = [inp.tile([P, S], f16, tag=f"qxT{ko}", name=f"qxT{ko}") for ko in range(KO)]
            vxT = [inp.tile([P, S], f16, tag=f"vxT{ko}", name=f"vxT{ko}") for ko in range(KO)]

            # ---- DMA emission ----
            # plain DMAs first, then ALL x-bar transposes strictly grouped
            # (a plain DMA interleaved into the transpose stream triggers
            # the xbar-mode serialization, ~5us per transpose issue), then
            # the late plain loads.
            for w, w_sb in ((wq, wq_sb), (wk, wk_sb)):
                nc.sync.dma_start(
                    w_sb[:], w.rearrange("(ko p) j -> p ko j", p=P)
                )
            nc.sync.dma_start(bq_sb[:], bq.rearrange("(t p) -> p t", p=P))
            nc.sync.dma_start(bk_sb[:], bk.rearrange("(t p) -> p t", p=P))
            nc.sync.dma_start(
                wv_sb[:], wv.rearrange("(ko p) j -> p ko j", p=P)
            )
            def xpose(x, xT, c0, c1):
                for ko in range(KO):
                    nc.sync.dma_start_transpose(xT[ko][:, c0:c1], x[ko][c0:c1])

            xpose(qx, qxT, 0, 512)
            xpose(kx, kxT, 0, 512)
            xpose(qx, qxT, 512, 1024)
            xpose(qx, qxT, 1024, 2048)
            xpose(kx, kxT, 512, 1024)
            xpose(vx, vxT, 0, 1024)
            xpose(kx, kxT, 1024, 1536)
            xpose(vx, vxT, 1024, 2048)
            xpose(kx, kxT, 1536, 2048)
            nc.sync.dma_start(wo_sb[:], wo.rearrange("(t p) m -> p t m", p=P))

            # ---- compute units ----
            def proj_unit(xT, w_sb, b_sb, dst, t, ic):
                ps = psm.tile([P, 512], f32, tag="ps", name="ps")[:, :IC]
                for ko in range(KO):
                    nc.tensor.matmul(
                        ps,
                        w_sb[:, ko, t * P : (t + 1) * P],
                        xT[ko][:, ic * IC : (ic + 1) * IC],
                        start=(ko == 0),
                        stop=(ko == KO - 1),
                    )
                nc.vector.tensor_scalar_add(
                    dst[t][:, ic * IC : (ic + 1) * IC],
                    ps,
                    b_sb[:, t : t + 1],
                )

            def vproj_unit(qb):
                ps = psm.tile([P, 512], f32, tag="ps", name="ps")
                for ko in range(KO):
                    nc.tensor.matmul(
                        ps[:, :JW],
                        vxT[ko][:, qb * P : (qb + 1) * P],
                        wv_sb[:, ko, :],
                        start=(ko == 0),
                        stop=(ko == KO - 1),
                    )
                nc.vector.tensor_copy(
                    vp4[:, qb, :, 0:HD],
                    ps[:, :JW].rearrange("p (h x) -> p h x", h=HPC),
                )

            etile = {}
            ottile = {}
            ontile = {}
            sttile = {}

            def pv_quantum(w, qb):
                p = w // NCC
                e = etile[(w, qb)]
                if qb == 0:
                    # 4 accumulation groups share each bank, so a start=True
                    # would zero the sibling groups' contributions: memset
                    # once, accumulate always (start=False).
                    for half in range(2):
                        ot = otp.tile(
                            [P, 2 * (2 * HD1)], f32,
                            tag=f"ot{half}", name=f"ot{half}",
                        )
                        nc.vector.memset(ot[:], 0.0)
                        ottile[(w, half)] = ot
                for cb in range(NCB):
                    ot = ottile[(w, cb // 2)]
                    base = (cb % 2) * (2 * HD1)
                    for i in range(2):
                        h = 2 * p + i
                        nc.tensor.matmul(
                            ot[:, base + i * HD1 : base + (i + 1) * HD1],
                            e[:, i * CC + cb * P : i * CC + (cb + 1) * P],
                            vp[:, qb, h * HD1 : (h + 1) * HD1],
                            start=False,
                            stop=(qb == NQB - 1),
                            skip_group_check=True,
                        )

            def post_unit(w, cb, tppool=None, stages=(0, 1)):
                p, cc = w // NCC, w % NCC
                ot = ottile[(w, cb // 2)]
                base = (cb % 2) * (2 * HD1)
                cpos = cc * CC + cb * P
                if 0 in stages:
                    zn = wrk.tile([P, 2], f32, tag="zn", name="zn", bufs=4)
                    for i in range(2):
                        nc.vector.reciprocal(
                            zn[:, i : i + 1],
                            ot[:, base + i * HD1 + HD : base + i * HD1 + HD + 1],
                        )
                    on = wrk.tile([P, P], f32, tag="on", name="on", bufs=4)
                    for i in range(2):
                        nc.vector.tensor_scalar_mul(
                            on[:, i * HD : (i + 1) * HD],
                            ot[:, base + i * HD1 : base + i * HD1 + HD],
                            zn[:, i : i + 1],
                        )
                    ontile[(w, cb)] = on
                if 1 not in stages:
                    return
                on = ontile.pop((w, cb))
                if tppool is None:
                    tp = psm.tile([P, 512], f32, tag="ps", name="tp")
                else:
                    tp = tppool.tile([P, 2 * CC], f32, tag="st", name="tp")
                nc.tensor.transpose(tp[:, 0:P], on[:], ident[:])
                if p == 0:
                    nc.vector.tensor_copy(opair0[:, cpos : cpos + P], tp[:, 0:P])
                else:
                    op1 = wrk.tile([P, P], f32r, tag="op1", name="op1", bufs=3)
                    nc.vector.tensor_copy(op1[:], tp[:, 0:P])
                    outproj(cpos, op1, drain=(tppool is not None))

            def outproj(cpos, op1, drain=False):
                for mch in range(2):
                    ps = psm.tile([P, 512], f32, tag="ps", name="ps")
                    for pp_, lhs in ((0, opair0[:, cpos : cpos + P]), (1, op1[:])):
                        nc.tensor.matmul(
                            ps[:, :MC],
                            lhs,
                            wo_sb[:, pp_, mch * MC : (mch + 1) * MC],
                            start=(pp_ == 0),
                            stop=(pp_ == 1),
                        )
                    ob = wrk.tile([P, MC], f32, tag="osb", name="osb", bufs=4)
                    if drain and mch == 0:
                        nc.scalar.copy(ob[:], ps[:, :MC])
                    else:
                        nc.vector.tensor_copy(ob[:], ps[:, :MC])
                    nc.sync.dma_start(
                        out[cpos : cpos + P, mch * MC : (mch + 1) * MC], ob[:]
                    )

            # ---- schedule ----
            sched = defaultdict(list)

            def at(w, slot, fn):
                sched[(w, slot)].append(fn)

            # background units (slots tuned to DMA arrival + PE-load
            # balance; see docstring).  proj unit granularity is 256 cols.
            def P_(xT, w_sb, b_sb, dst, t, ic):
                return lambda: proj_unit(xT, w_sb, b_sb, dst, t, ic)

            qp0 = lambda ic: P_(qxT, wq_sb, bq_sb, qpt, 0, ic)
            kp0 = lambda ic: P_(kxT, wk_sb, bk_sb, kpt, 0, ic)
            qp1 = lambda ic: P_(qxT, wq_sb, bq_sb, qpt, 1, ic)
            kp1 = lambda ic: P_(kxT, wk_sb, bk_sb, kpt, 1, ic)
            for slot, ic in zip((1, 2, 4, 5, 7, 8), range(2, 8)):
                at(0, slot, qp0(ic))           # qpt0 rest
            at(0, 10, kp0(2)); at(0, 11, kp0(3))      # kpt0 cc1
            for slot, ic in zip((3, 6, 9, 12), range(4)):
                at(0, slot, qp1(ic))           # qpt1 first half (qx all in)
            at(1, 10, kp0(4)); at(1, 11, kp0(5))      # kpt0 cc2
            at(2, 2, kp0(6)); at(2, 5, kp0(7))        # kpt0 cc3
            for slot, ic in zip((1, 2, 4, 5), range(4, 8)):
                at(3, slot, qp1(ic))           # qpt1 second half
            at(3, 7, kp1(0)); at(3, 8, kp1(1))        # kpt1 cc0
            at(4, 1, kp1(2)); at(4, 4, kp1(3))        # kpt1 cc1
            at(5, 1, kp1(4)); at(5, 4, kp1(5))        # kpt1 cc2
            at(6, 1, kp1(6)); at(6, 4, kp1(7))        # kpt1 cc3

            # PV quanta lag one full window behind the exp stream; vproj
            # paired just before the first quantum needing it.
            pv_slots = [0, 0, 1, 1, 2, 2, 3, 3, 4, 5, 6, 7, 8, 9, 10, 11]
            at(0, 14, lambda: vproj_unit(0))
            at(0, 15, lambda: vproj_unit(1))
            for qb in range(2, NQB):
                at(1, pv_slots[qb], lambda qb=qb: vproj_unit(qb))
            # windows 0..5: quanta(w) in window w+1 at pv_slots, posts at
            # 12-15.  End-game compression: quanta(w5) packed 2/slot early
            # in w6, posts(w5) at w6 slots 8-11, quanta(w6) at w6 tail,
            # posts(w6) at w7 slots 0-3, quanta(w7) inside w7 as e tiles
            # land, posts(w7) stage-interleaved in the drain.
            for w in range(W - 2):
                for qb in range(NQB):
                    at(w + 1, pv_slots[qb], lambda w=w, qb=qb: pv_quantum(w, qb))
                if w < W - 3:
                    for cb in range(NCB):
                        at(w + 1, 12 + cb, lambda w=w, cb=cb: post_unit(w, cb))
            for cb in range(NCB):
                at(W - 2, 11, lambda cb=cb: post_unit(W - 3, cb))
            w6_slots = [11, 11, 11, 11, 12, 12, 12, 12, 13, 13, 13, 13, 14, 14, 15, 15]
            for qb in range(NQB):
                at(W - 2, w6_slots[qb], lambda qb=qb: pv_quantum(W - 2, qb))
            for cb in range(NCB):
                at(W - 1, cb, lambda cb=cb: post_unit(W - 2, cb))
            w7_slots = [4, 4, 5, 5, 6, 6, 7, 7, 8, 9, 10, 11, 12, 13, 14, 15]
            for qb in range(NQB):
                at(W - 1, w7_slots[qb], lambda qb=qb: pv_quantum(W - 1, qb))
            for cb in range(NCB):
                at(W, 12 + cb, lambda cb=cb: post_unit(W - 1, cb, stages=(0,)))
            for cb in range(NCB):
                at(W, 15, lambda cb=cb: post_unit(W - 1, cb, tppool=stp, stages=(1,)))

            # ---- emission (scores lead their exp by one slot) ----
            def emit_score(w, qb):
                p, cc = w // NCC, w % NCC
                st = stp.tile([P, 2 * CC], f32, tag="st", name="st")
                for i in range(2):
                    r0 = i * HD
                    nc.tensor.matmul(
                        st[:, i * CC : (i + 1) * CC],
                        qpt[p][r0 : r0 + HD, qb * P : (qb + 1) * P],
                        kpt[p][r0 : r0 + HD, cc * CC : (cc + 1) * CC],
                        start=True,
                        stop=True,
                    )
                sttile[(w, qb)] = st

            proj_unit(qxT, wq_sb, bq_sb, qpt, 0, 0)
            proj_unit(kxT, wk_sb, bk_sb, kpt, 0, 0)
            proj_unit(kxT, wk_sb, bk_sb, kpt, 0, 1)
            proj_unit(qxT, wq_sb, bq_sb, qpt, 0, 1)
            emit_score(0, 0)
            for w in range(W):
                for qb in range(NQB):
                    st = sttile.pop((w, qb))
                    e = ep.tile([P, 2 * CC], bf16, tag="e", name="e")
                    nc.scalar.activation(e[:], st[:], EXP)
                    etile[(w, qb)] = e
                    if qb + 1 < NQB:
                        emit_score(w, qb + 1)
                    elif w + 1 < W:
                        emit_score(w + 1, 0)
                    for fn in sched.pop((w, qb), []):
                        fn()
            # drain
            for slot in range(NQB):
                for fn in sched.pop((W, slot), []):
                    fn()
            assert not sched, f"unscheduled work: {list(sched)}"
    return nc


_NC_CACHE = {}


def _get_program(S=S_FULL, repeat=1):
    key = (S, repeat)
    if key not in _NC_CACHE:
        import concourse.bacc as bacc

        nc = bacc.Bacc(trn_type="TRN2", target_bir_lowering=False)
        build(nc, S, repeat)
        nc.compile()
        _NC_CACHE[key] = nc
    return _NC_CACHE[key]


def _slab_major(x):
    """[S, DM] -> [DM//128, S, 128] fp16, each 128-feature slab contiguous."""
    s, dm = x.shape
    return np.ascontiguousarray(
        x.reshape(s, dm // 128, 128).transpose(1, 0, 2)
    ).astype(np.float16)


def make_in_maps(inputs, S=S_FULL):
    """Per-core input dicts. Core c: batch c//4, head group c%4."""
    f16 = np.float16
    k, q, v = inputs["k"], inputs["q"], inputs["v"]
    in_maps = []
    for c in range(NCORES):
        b, g = c // 4, c % 4
        j0, j1 = g * JW, (g + 1) * JW
        in_maps.append(
            {
                "kx": _slab_major(k[b, :S]),
                "qx": _slab_major(q[b, :S]),
                "vx": _slab_major(v[b, :S]),
                "wk": np.ascontiguousarray(inputs["Wk"][:, j0:j1]).astype(f16),
                "wq": np.ascontiguousarray(inputs["Wq"][:, j0:j1]).astype(f16),
                "wv": np.ascontiguousarray(inputs["Wv"][:, j0:j1]).astype(f16),
                "wo": np.ascontiguousarray(inputs["Wo"][j0:j1, :], dtype=np.float32),
                "bk": np.ascontiguousarray(inputs["bk"][j0:j1], dtype=np.float32),
                "bq": np.ascontiguousarray(inputs["bq"][j0:j1], dtype=np.float32),
            }
        )
    return in_maps


def gather(results, inputs, S=S_FULL):
    out = np.zeros((B, S, DM), np.float32)
    for c in range(NCORES):
        out[c // 4] += results[c]["out"]
    # bias terms: softmax rows sum to 1, so the v-bias passes through
    # attention unchanged -> contributes bv @ Wo; plus bo.
    corr = (
        np.asarray(inputs["bv"], np.float32) @ np.asarray(inputs["Wo"], np.float32)
        + np.asarray(inputs["bo"], np.float32)
    )
    return out + corr[None, None, :]


def kernel(**inputs):
    inputs = {k: np.asarray(v) for k, v in inputs.items()}
    nc = _get_program()
    in_maps = make_in_maps(inputs)
    from concourse import bass_utils

    res = bass_utils.run_bass_kernel_spmd(
        nc, in_maps, core_ids=list(range(NCORES))
    )
    return gather(res.results, inputs)
